# revision 12
# baseline (speedup 1.0000x reference)
"""Trainium2 Bass kernel for nn_AttentionConv (dense_transformer).

Sharding: data-parallel over batch — 8 NeuronCores, one batch image each.

Per-core dataflow (T=3136 tokens = 56x56, C=384, 6 heads x 64):
  - x shipped pre-transposed from host as xT [C, 58*58] bf16 (zero-padded).
  - Q path: depthwise 3x3 conv + BN tap-accumulated on DVE (first row-slab,
    so band 0 is ready early) + GPSIMD (remaining slabs, an otherwise-idle
    engine) via scalar_tensor_tensor; BN affine folded into tap scales and
    bias on host. Q projection on PE (softmax scale folded into wq).
  - K/V path: stride-2 depthwise conv as 9 diagonal-stationary PE matmuls
    (per-channel tap scale on the diagonal), BN bias at PSUM evacuation,
    then a plain [C->C] projection on PE.
  - Attention in 512-wide q bands, heads processed in PAIRS (2p, 2p+1 of
    the same 64-partition halves of ctile p):
      scores^T [t, q] for both heads land in ONE [128, 1024] f32 PSUM tile
      as two row-tiled matmuls (K=64 each, rows 0:64 / 64:128 of the PE
      array -> concurrent execution, ~2x scores throughput);
      ONE exp [tsz, 1024] on ACT covers the pair (ACT is the attention
      bottleneck; half the instruction count vs per-head exp);
      o^T accumulated per head via [vh | ones] stationaries into a shared
      [65, 1024] PSUM tile (A cols 0:512, B cols 512:1024) - the ones
      column yields both denominators on PSUM row 64, so the pair shares
      ONE norm chain (den row -> DRAM fold -> reciprocal -> broadcast).
  - Output projection + next-band Q projection interleaved into the band
    loop to fill PE while ACT runs exp. Tiny 64-wide tail band runs last
    (t-grouped scores -> 3 exps, one shared norm chain) so the serial
    tail is short. b_last added on host.
"""
import sys

sys.path.insert(0, '/opt/trn_rl_repo')

import numpy as np

DIM = 384
HEADS = 6
D = 64
S = 56           # stride-1 spatial side
S2 = 28          # stride-2 spatial side
T = S * S        # 3136
T2 = S2 * S2     # 784
EPS = 1e-5
SCALE = DIM ** -0.5
NCORES = 8
CT = DIM // 128          # 3 channel tiles
NTT = (T2 + 127) // 128  # 7 kv t-tiles (last = 16 rows)
W = 512                  # attention q band width
NB = 6                   # full bands; tail band is 64 wide
LCH = 448                # q-projection chunk (8 h-rows)

TAPS = [(dy, dx) for dy in (-1, 0, 1) for dx in (-1, 0, 1)]  # k=(dy+1)*3+(dx+1)


def build_program():
    import concourse.mybir as mybir
    from concourse import bacc
    from concourse.tile import TileContext

    dt = mybir.dt
    AF = mybir.ActivationFunctionType
    ALU = mybir.AluOpType

    nc = bacc.Bacc()

    SP = S + 2
    xT = nc.dram_tensor("xT", [DIM, SP * SP], dt.bfloat16,
                        kind="ExternalInput")
    qcp = nc.dram_tensor("qcp", [DIM, 10], dt.float32, kind="ExternalInput")
    wqt = nc.dram_tensor("wqt", [DIM, DIM], dt.bfloat16, kind="ExternalInput")
    wkvt = nc.dram_tensor("wkvt", [DIM, 2, DIM], dt.bfloat16,
                          kind="ExternalInput")
    kvs = nc.dram_tensor("kvs", [DIM, 18], dt.float32, kind="ExternalInput")
    kvb = nc.dram_tensor("kvb", [DIM, 2], dt.float32, kind="ExternalInput")
    wlt = nc.dram_tensor("wlt", [DIM, DIM], dt.bfloat16, kind="ExternalInput")
    idin = nc.dram_tensor("idin", [128, 128], dt.bfloat16, kind="ExternalInput")
    out = nc.dram_tensor("out", [T, DIM], dt.float32, kind="ExternalOutput")

    with TileContext(nc) as tc:
        with (
            tc.tile_pool(name="const", bufs=1) as cpool,
            tc.tile_pool(name="ework", bufs=3) as epool,
            tc.tile_pool(name="rrep", bufs=2) as rpool,
            tc.tile_pool(name="ostg", bufs=2) as opool,
            tc.tile_pool(name="psA", bufs=2, space="PSUM") as psA,
            tc.tile_pool(name="psO", bufs=2, space="PSUM") as psO,
            tc.tile_pool(name="dram", bufs=2, space="DRAM") as dpool,
        ):
            # ---------------- Phase 0: loads ----------------
            xT_sb = cpool.tile([128, CT, SP, SP], dt.bfloat16)
            qcp_sb = cpool.tile([128, CT, 10], dt.float32)
            wqt_sb = cpool.tile([128, CT, DIM], dt.bfloat16)
            dk_sb = cpool.tile([128, 9 * CT, 128], dt.bfloat16)
            dv_sb = cpool.tile([128, 9 * CT, 128], dt.bfloat16)
            kf_sb = cpool.tile([128, CT, T2], dt.bfloat16)
            vf_sb = cpool.tile([128, CT, T2], dt.bfloat16)
            kvb_sb = cpool.tile([128, CT, 2], dt.float32)
            wlt_sb = cpool.tile([128, CT, DIM], dt.bfloat16)
            ident = cpool.tile([128, 128], dt.bfloat16)
            wkvt_sb = cpool.tile([128, CT, 2, DIM], dt.bfloat16)
            kvs_sb = cpool.tile([128, CT, 18], dt.float32)

            def csl(c):
                return slice(c * 128, (c + 1) * 128)

            nc.sync.dma_start(ident[:], idin[:])
            for c in range(CT):
                nc.sync.dma_start(kvs_sb[:, c, :], kvs[csl(c), :])
                nc.sync.dma_start(qcp_sb[:, c, :], qcp[csl(c), :])
                nc.sync.dma_start(
                    xT_sb[:, c, :, :],
                    xT[csl(c), :].rearrange("p (h w) -> p h w", w=SP))
            for c in range(CT):
                nc.sync.dma_start(wkvt_sb[:, c, :, :], wkvt[csl(c), :, :])
                nc.sync.dma_start(wqt_sb[:, c, :], wqt[csl(c), :])
                nc.sync.dma_start(kvb_sb[:, c, :], kvb[csl(c), :])
                nc.sync.dma_start(wlt_sb[:, c, :], wlt[csl(c), :])

            # K diag stationaries on DVE (needed first); V's on ACT (idle
            # until attention, fills the K-conv stretch).
            for c in range(CT):
                for k in range(9):
                    nc.vector.tensor_scalar(
                        out=dk_sb[:, k * CT + c, :], in0=ident[:],
                        scalar1=kvs_sb[:, c, k:k + 1], scalar2=0.0,
                        op0=ALU.mult, op1=ALU.add)
                    nc.scalar.activation(
                        dv_sb[:, k * CT + c, :], ident[:],
                        AF.Copy, scale=kvs_sb[:, c, 9 + k:10 + k])

            # persistent activations
            q_feat = cpool.tile([128, CT, T], dt.bfloat16)
            qh_sb = cpool.tile([128, CT, T], dt.bfloat16)
            kh_sb = cpool.tile([128, CT, T2], dt.bfloat16)
            vh_sb = cpool.tile([128, CT, T2], dt.bfloat16)
            vhT_sb = cpool.tile([128, NTT, HEADS * 65], dt.bfloat16)
            o_sb = cpool.tile([128, CT, T], dt.bfloat16)
            den_scr = cpool.tile([128, 2 * W], dt.float32)
            den_fold = cpool.tile([128, 2 * W // 8], dt.float32)
            r_fold = cpool.tile([128, 2 * W // 8], dt.float32)

            # ---- Phase 1: Q depthwise conv + BN, tap accumulation in
            # q_feat (bf16, in place). Row-slab r0:r1; engine eng.
            def q_conv_slab(eng, c, r0, r1):
                x3 = xT_sb[:, c, :, :]  # [128, 58, 58] zero-padded
                dst = q_feat[:, c, r0 * S:r1 * S].rearrange(
                    "p (h w) -> p h w", w=S)
                for k in range(9):
                    dy, dx = TAPS[k]
                    xs = x3[:, 1 + dy + r0:1 + dy + r1, 1 + dx:1 + dx + S]
                    if k == 0:
                        eng.tensor_scalar(
                            out=dst, in0=xs,
                            scalar1=qcp_sb[:, c, 0:1],
                            scalar2=qcp_sb[:, c, 9:10],
                            op0=ALU.mult, op1=ALU.add)
                    else:
                        eng.scalar_tensor_tensor(
                            out=dst, in0=xs, scalar=qcp_sb[:, c, k:k + 1],
                            in1=dst, op0=ALU.mult, op1=ALU.add)

            # rows 0:24 now (runs on DVE during the PE-bound KV phase);
            # rows 24+8b:32+8b burst at the start of band b (gpsimd cannot
            # run TensorScalarPtr ops on this ISA, so DVE carries the conv).
            for c in range(CT):
                q_conv_slab(nc.vector, c, 0, 24)

            # ------------- Phase 2: K/V stride-2 conv + projection ----------
            def kv_conv(d_sb, f_sb, bias_col):
                for c in range(CT):
                    x5 = xT_sb[:, c, :, :].rearrange(
                        "p (h sy) (w sx) -> p h sy w sx", sy=2, sx=2)
                    for ha, hb in ((0, 14), (14, 28)):
                        ps = psA.tile([128, 2 * W], dt.float32, tag="psA")
                        for k in range(9):
                            dy, dx = TAPS[k]
                            hoff, sy = ((0, 0) if dy == -1 else
                                        (0, 1) if dy == 0 else (1, 0))
                            woff, sx = ((0, 0) if dx == -1 else
                                        (0, 1) if dx == 0 else (1, 0))
                            nc.tensor.matmul(
                                ps[:, 0:(hb - ha) * S2],
                                d_sb[:, k * CT + c, :],
                                x5[:, ha + hoff:hb + hoff, sy,
                                   woff:woff + S2, sx],
                                start=(k == 0), stop=(k == 8))
                        nc.vector.tensor_scalar_add(
                            f_sb[:, c, ha * S2:hb * S2],
                            ps[:, 0:14 * S2],
                            kvb_sb[:, c, bias_col:bias_col + 1])

            def kv_proj(f_sb, dst_sb, wcol):
                for ot in range(CT):
                    osl = slice(ot * 128, (ot + 1) * 128)
                    for ha, hb in ((0, 14), (14, 28)):
                        ps = psA.tile([128, 2 * W], dt.float32, tag="psA")
                        for c in range(CT):
                            nc.tensor.matmul(
                                ps[:, 0:(hb - ha) * S2],
                                wkvt_sb[:, c, wcol, osl],
                                f_sb[:, c, ha * S2:hb * S2],
                                start=(c == 0), stop=(c == CT - 1))
                        nc.vector.tensor_copy(
                            dst_sb[:, ot, ha * S2:hb * S2],
                            ps[:, 0:14 * S2])

            kv_conv(dk_sb, kf_sb, 0)
            kv_proj(kf_sb, kh_sb, 0)
            kv_conv(dv_sb, vf_sb, 1)
            kv_proj(vf_sb, vh_sb, 1)

            # -------- Phase 3: vh^T [t, (head, 65)] with ones columns -------
            v4 = vhT_sb[:].rearrange("p n (h c) -> p n h c", c=65)
            nc.vector.memset(vhT_sb[:], 1.0)
            for tt in range(NTT):
                tsz = min(128, T2 - tt * 128)
                for ot in range(CT):
                    pst = psO.tile([128, 2 * W], dt.bfloat16, tag="psO")
                    nc.tensor.transpose(
                        pst[0:tsz, 0:128],
                        vh_sb[:, ot, tt * 128:tt * 128 + tsz],
                        ident[:])
                    nc.vector.tensor_copy(
                        v4[0:tsz, tt, 2 * ot:2 * ot + 2, 0:64],
                        pst[0:tsz, 0:128].rearrange("p (h c) -> p h c", c=64))

            # ---------------- Phase 4: Q projection chunks ------------------
            def qproj_unit(lc, ot):
                lsl = slice(lc * LCH, (lc + 1) * LCH)
                osl = slice(ot * 128, (ot + 1) * 128)
                ps = psA.tile([128, 2 * W], dt.float32, tag="psA")
                for c in range(CT):
                    nc.tensor.matmul(
                        ps[:, 0:LCH], wqt_sb[:, c, osl], q_feat[:, c, lsl],
                        start=(c == 0), stop=(c == CT - 1))
                nc.vector.tensor_copy(qh_sb[:, ot, lsl], ps[:, 0:LCH])

            for lc in (0, 1, 2):
                for ot in range(CT):
                    qproj_unit(lc, ot)

            # ---------------- Phase 5: attention ----------------
            def oproj_tile(lpos, lsz):
                ps = psA.tile([128, 2 * W], dt.float32, tag="psA")
                for c in range(CT):
                    nc.tensor.matmul(
                        ps[0:lsz, 0:DIM], o_sb[:, c, lpos:lpos + lsz],
                        wlt_sb[:, c, :],
                        start=(c == 0), stop=(c == CT - 1))
                ostage = opool.tile([128, DIM], dt.float32, tag="ostage")
                nc.vector.tensor_copy(ostage[0:lsz, :], ps[0:lsz, 0:DIM])
                nc.sync.dma_start(out[lpos:lpos + lsz, :], ostage[0:lsz, :])

            def norm_chain(ps_o, WW):
                """den row -> fold via DRAM -> recip -> broadcast [64, WW]."""
                fw = 8
                fp = WW // fw
                den_dr = dpool.tile([2 * W], dt.float32, tag="dd")
                r_dr = dpool.tile([2 * W], dt.float32, tag="rd")
                nc.vector.tensor_copy(den_scr[64:65, 0:WW],
                                      ps_o[64:65, 0:WW])
                nc.sync.dma_start(den_dr[None, 0:WW], den_scr[64:65, 0:WW])
                nc.sync.dma_start(
                    den_fold[0:fp, 0:fw],
                    den_dr[0:WW].rearrange("(p f) -> p f", f=fw))
                nc.vector.reciprocal(r_fold[0:fp, 0:fw],
                                     den_fold[0:fp, 0:fw])
                nc.sync.dma_start(
                    r_dr[0:WW].rearrange("(p f) -> p f", f=fw),
                    r_fold[0:fp, 0:fw])
                r_rep = rpool.tile([64, 2 * W], dt.float32, tag="r_rep")
                nc.sync.dma_start(r_rep[0:64, 0:WW],
                                  r_dr[None, 0:WW].to_broadcast([64, WW]))
                return r_rep

            # interleave queue: (kind, args) units issued between pairs
            def run_units(units, n):
                for _ in range(n):
                    if not units:
                        return
                    kind, args = units.pop(0)
                    if kind == 'oproj':
                        oproj_tile(*args)
                    else:
                        qproj_unit(*args)

            units = []
            for bi in range(NB):
                qs = bi * W
                # conv slab for q-proj chunk lc=bi+3 (consumed next band)
                if bi < 4:
                    for c in range(CT):
                        q_conv_slab(nc.vector, c, 24 + 8 * bi, 32 + 8 * bi)
                # stage next-band q-projection + previous band's out-proj
                if bi + 3 <= 6:
                    units += [('qproj', (bi + 3, ot)) for ot in range(CT)]
                if bi > 0:
                    pq = (bi - 1) * W
                    units += [('oproj', (pq + i * 128, 128)) for i in range(4)]

                for p in range(CT):  # head pair (2p, 2p+1), ctile p
                    hA, hB = 2 * p, 2 * p + 1
                    ps_o = psO.tile([128, 2 * W], dt.float32, tag="psO")
                    for tt in range(NTT):
                        tsz = min(128, T2 - tt * 128)
                        tslc = slice(tt * 128, tt * 128 + tsz)
                        ps_s = psA.tile([128, 2 * W], dt.float32, tag="psA")
                        nc.tensor.matmul(
                            ps_s[0:tsz, 0:W],
                            kh_sb[0:64, p, tslc],
                            qh_sb[0:64, p, qs:qs + W],
                            start=True, stop=True)
                        nc.tensor.matmul(
                            ps_s[0:tsz, W:2 * W],
                            kh_sb[64:128, p, tslc],
                            qh_sb[64:128, p, qs:qs + W],
                            start=True, stop=True)
                        e = epool.tile([128, 2 * W], dt.bfloat16, tag="e")
                        nc.scalar.activation(e[0:tsz, 0:2 * W],
                                             ps_s[0:tsz, 0:2 * W], AF.Exp)
                        nc.tensor.matmul(
                            ps_o[0:65, 0:W],
                            vhT_sb[0:tsz, tt, hA * 65:hA * 65 + 65],
                            e[0:tsz, 0:W],
                            start=(tt == 0), stop=(tt == NTT - 1))
                        nc.tensor.matmul(
                            ps_o[0:65, W:2 * W],
                            vhT_sb[0:tsz, tt, hB * 65:hB * 65 + 65],
                            e[0:tsz, W:2 * W],
                            start=(tt == 0), stop=(tt == NTT - 1))
                    r_rep = norm_chain(ps_o, 2 * W)
                    nc.vector.tensor_tensor(
                        out=o_sb[0:64, p, qs:qs + W],
                        in0=ps_o[0:64, 0:W],
                        in1=r_rep[0:64, 0:W], op=ALU.mult)
                    nc.vector.tensor_tensor(
                        out=o_sb[64:128, p, qs:qs + W],
                        in0=ps_o[0:64, W:2 * W],
                        in1=r_rep[0:64, W:2 * W], op=ALU.mult)
                    # oproj units read the previous band's o_sb, whose last
                    # evac lands a few us into this band — consume units
                    # only after pairs 1 and 2 so PE never queues a waiting
                    # matmul ahead of live scores work.
                    if p == 1:
                        run_units(units, 3)
                    elif p == 2:
                        run_units(units, 4)

            # ---------------- Phase 6: 64-wide tail band ----------------
            # t-grouped scores: per pair ONE [tsz, 896] psum tile (cols
            # tt*128 + {0:64 A, 64:128 B}) -> ONE exp; shared ps_o
            # [65, 6*64] with one norm chain for all heads.
            qs, WT = NB * W, T - NB * W  # 3072, 64
            ps_o = psO.tile([128, 2 * W], dt.float32, tag="psO")
            for p in range(CT):
                hA, hB = 2 * p, 2 * p + 1
                for hh, h in ((0, hA), (1, hB)):
                    hsl = slice(64 * hh, 64 * hh + 64)
                    ps_s = psA.tile([128, 2 * W], dt.float32, tag="psA")
                    for tt in range(NTT):
                        tsz = min(128, T2 - tt * 128)
                        tslc = slice(tt * 128, tt * 128 + tsz)
                        nc.tensor.matmul(
                            ps_s[0:tsz, tt * 128:tt * 128 + WT],
                            kh_sb[hsl, p, tslc],
                            qh_sb[hsl, p, qs:qs + WT],
                            start=True, stop=True)
                    e = epool.tile([128, 2 * W], dt.bfloat16, tag="e")
                    nc.scalar.activation(e[0:128, 0:(NTT - 1) * 128],
                                         ps_s[0:128, 0:(NTT - 1) * 128],
                                         AF.Exp)
                    nc.scalar.activation(
                        e[0:16, (NTT - 1) * 128:NTT * 128],
                        ps_s[0:16, (NTT - 1) * 128:NTT * 128], AF.Exp)
                    # six head groups share one PSUM zero region (2KB bank)
                    # so each accumulation group fully closes before the
                    # next starts
                    for tt in range(NTT):
                        tsz = min(128, T2 - tt * 128)
                        nc.tensor.matmul(
                            ps_o[0:65, h * WT:h * WT + WT],
                            vhT_sb[0:tsz, tt, h * 65:h * 65 + 65],
                            e[0:tsz, tt * 128:tt * 128 + WT],
                            start=(tt == 0), stop=(tt == NTT - 1))
                # last 512-band's out-projection rides the tail band
                pq = (NB - 1) * W
                if p == 1:
                    oproj_tile(pq, 128)
                    oproj_tile(pq + 128, 128)
                elif p == 2:
                    oproj_tile(pq + 256, 128)
                    oproj_tile(pq + 384, 128)
            r_rep = norm_chain(ps_o, HEADS * WT)
            for h in range(HEADS):
                nc.vector.tensor_tensor(
                    out=o_sb[64 * (h % 2):64 * (h % 2) + 64, h // 2,
                             qs:qs + WT],
                    in0=ps_o[0:64, h * WT:h * WT + WT],
                    in1=r_rep[0:64, h * WT:h * WT + WT], op=ALU.mult)
            run_units(units, 99)
            oproj_tile(qs, WT)

    nc.compile()
    return nc


_CACHE = {}


def _prep_weights(inputs):
    import ml_dtypes
    bf16 = ml_dtypes.bfloat16
    f32 = np.float32

    def bn_fold(prefix):
        a = (np.asarray(inputs[f'bn{prefix}_s'], f32)
             / np.sqrt(np.asarray(inputs[f'bn{prefix}_v'], f32) + EPS))
        b = (np.asarray(inputs[f'bn{prefix}_b'], f32)
             - np.asarray(inputs[f'bn{prefix}_m'], f32) * a)
        return a.astype(f32), b.astype(f32)

    aq, bq = bn_fold('q')
    ak, bk = bn_fold('k')
    av, bv = bn_fold('v')

    conv_q = np.asarray(inputs['conv_q'], f32)[:, 0].reshape(DIM, 9)
    conv_k = np.asarray(inputs['conv_k'], f32)[:, 0].reshape(DIM, 9)
    conv_v = np.asarray(inputs['conv_v'], f32)[:, 0].reshape(DIM, 9)
    wq = np.asarray(inputs['wq'], f32)
    wk = np.asarray(inputs['wk'], f32)
    wv = np.asarray(inputs['wv'], f32)
    wl = np.asarray(inputs['w_last'], f32)

    qcp = np.zeros((DIM, 10), f32)
    qcp[:, :9] = conv_q * aq[:, None]
    qcp[:, 9] = bq

    wqt = np.ascontiguousarray((wq * SCALE).T).astype(bf16)  # [c, o]

    wkvt = np.stack([wk.T, wv.T], axis=1).astype(bf16)  # [c, {k,v}, o]
    kvs = np.concatenate([conv_k * ak[:, None], conv_v * av[:, None]],
                         axis=1).astype(f32)            # [c, 18]
    kvb = np.stack([bk, bv], axis=1).astype(f32)        # [c, 2]
    wlt = np.ascontiguousarray(wl.T).astype(bf16)
    idin = np.eye(128, dtype=bf16)
    return qcp, wqt, wkvt, kvs, kvb, wlt, idin


def _prep_x(xb):
    """[T, C] f32 -> zero-padded transposed [C, 58*58] bf16."""
    import ml_dtypes
    pad = np.zeros((DIM, S + 2, S + 2), np.float32)
    pad[:, 1:1 + S, 1:1 + S] = xb.T.reshape(DIM, S, S)
    return pad.reshape(DIM, (S + 2) * (S + 2)).astype(ml_dtypes.bfloat16)


def kernel(**inputs):
    from concourse.bass_utils import run_bass_kernel_spmd

    if 'nc' not in _CACHE:
        _CACHE['nc'] = build_program()
    nc = _CACHE['nc']

    qcp, wqt, wkvt, kvs, kvb, wlt, idin = _prep_weights(inputs)
    x = np.asarray(inputs['x'], np.float32)  # [8, T, C]
    B = x.shape[0]

    in_maps = []
    for b in range(B):
        in_maps.append({
            'xT': _prep_x(x[b]), 'qcp': qcp, 'wqt': wqt, 'wkvt': wkvt,
            'kvs': kvs, 'kvb': kvb, 'wlt': wlt, 'idin': idin,
        })

    res = run_bass_kernel_spmd(nc, in_maps, list(range(NCORES)))
    outs = np.stack([np.asarray(res.results[b]['out']) for b in range(B)],
                    axis=0)
    outs = outs + np.asarray(inputs['b_last'], np.float32)[None, None, :]
    return outs.astype(np.float32)


# revision 18
# speedup vs baseline: 1.0267x; 1.0267x over previous
"""Trainium2 Bass kernel for nn_AttentionConv (dense_transformer).

Sharding: data-parallel over batch — 8 NeuronCores, one batch image each.

Per-core dataflow (T=3136 tokens = 56x56, C=384, 6 heads x 64):
  - x shipped pre-transposed from host as xT [C, 58*58] bf16 (zero-padded).
  - Q path: depthwise 3x3 conv + BN tap-accumulated on DVE (first row-slab,
    so band 0 is ready early) + GPSIMD (remaining slabs, an otherwise-idle
    engine) via scalar_tensor_tensor; BN affine folded into tap scales and
    bias on host. Q projection on PE (softmax scale folded into wq).
  - K/V path: stride-2 depthwise conv as 9 diagonal-stationary PE matmuls
    (per-channel tap scale on the diagonal), BN bias at PSUM evacuation,
    then a plain [C->C] projection on PE.
  - Attention in 512-wide q bands, heads processed in PAIRS (2p, 2p+1 of
    the same 64-partition halves of ctile p):
      scores^T [t, q] for both heads land in ONE [128, 1024] f32 PSUM tile
      as two row-tiled matmuls (K=64 each, rows 0:64 / 64:128 of the PE
      array -> concurrent execution, ~2x scores throughput);
      ONE exp [tsz, 1024] on ACT covers the pair (ACT is the attention
      bottleneck; half the instruction count vs per-head exp);
      o^T accumulated per head via [vh | ones] stationaries into a shared
      [65, 1024] PSUM tile (A cols 0:512, B cols 512:1024) - the ones
      column yields both denominators on PSUM row 64, so the pair shares
      ONE norm chain (den row -> DRAM fold -> reciprocal -> broadcast).
  - Output projection + next-band Q projection interleaved into the band
    loop to fill PE while ACT runs exp. Tiny 64-wide tail band runs last
    (t-grouped scores -> 3 exps, one shared norm chain) so the serial
    tail is short. b_last added on host.
"""
import sys

sys.path.insert(0, '/opt/trn_rl_repo')

import numpy as np

DIM = 384
HEADS = 6
D = 64
S = 56           # stride-1 spatial side
S2 = 28          # stride-2 spatial side
T = S * S        # 3136
T2 = S2 * S2     # 784
EPS = 1e-5
SCALE = DIM ** -0.5
NCORES = 8
CT = DIM // 128          # 3 channel tiles
NTT = (T2 + 127) // 128  # 7 kv t-tiles (last = 16 rows)
W = 512                  # attention q band width
NB = 6                   # full bands; tail band is 64 wide
LCH = 448                # q-projection chunk (8 h-rows)

TAPS = [(dy, dx) for dy in (-1, 0, 1) for dx in (-1, 0, 1)]  # k=(dy+1)*3+(dx+1)


def build_program():
    import concourse.mybir as mybir
    from concourse import bacc
    from concourse.tile import TileContext

    dt = mybir.dt
    AF = mybir.ActivationFunctionType
    ALU = mybir.AluOpType

    nc = bacc.Bacc()

    SP = S + 2
    xT = nc.dram_tensor("xT", [DIM, SP * SP], dt.bfloat16,
                        kind="ExternalInput")
    qcp = nc.dram_tensor("qcp", [DIM, 10], dt.float32, kind="ExternalInput")
    wqt = nc.dram_tensor("wqt", [DIM, DIM], dt.bfloat16, kind="ExternalInput")
    wkvt = nc.dram_tensor("wkvt", [DIM, 2, DIM], dt.bfloat16,
                          kind="ExternalInput")
    kvs = nc.dram_tensor("kvs", [DIM, 18], dt.float32, kind="ExternalInput")
    kvb = nc.dram_tensor("kvb", [DIM, 2], dt.float32, kind="ExternalInput")
    wlt = nc.dram_tensor("wlt", [DIM, DIM], dt.bfloat16, kind="ExternalInput")
    idin = nc.dram_tensor("idin", [128, 128], dt.bfloat16, kind="ExternalInput")
    out = nc.dram_tensor("out", [T, DIM], dt.float32, kind="ExternalOutput")

    with TileContext(nc) as tc:
        with (
            tc.tile_pool(name="const", bufs=1) as cpool,
            tc.tile_pool(name="ework", bufs=3) as epool,
            tc.tile_pool(name="rrep", bufs=2) as rpool,
            tc.tile_pool(name="ostg", bufs=2) as opool,
            tc.tile_pool(name="psA", bufs=2, space="PSUM") as psA,
            tc.tile_pool(name="psO", bufs=2, space="PSUM") as psO,
            tc.tile_pool(name="dram", bufs=2, space="DRAM") as dpool,
        ):
            # ---------------- Phase 0: loads ----------------
            xT_sb = cpool.tile([128, CT, SP, SP], dt.bfloat16)
            qcp_sb = cpool.tile([128, CT, 10], dt.float32)
            wqt_sb = cpool.tile([128, CT, DIM], dt.bfloat16)
            dk_sb = cpool.tile([128, 9 * CT, 128], dt.bfloat16)
            dv_sb = cpool.tile([128, 9 * CT, 128], dt.bfloat16)
            kf_sb = cpool.tile([128, CT, T2], dt.bfloat16)
            vf_sb = cpool.tile([128, CT, T2], dt.bfloat16)
            kvb_sb = cpool.tile([128, CT, 2], dt.float32)
            wlt_sb = cpool.tile([128, CT, DIM], dt.bfloat16)
            ident = cpool.tile([128, 128], dt.bfloat16)
            wkvt_sb = cpool.tile([128, CT, 2, DIM], dt.bfloat16)
            kvs_sb = cpool.tile([128, CT, 18], dt.float32)

            def csl(c):
                return slice(c * 128, (c + 1) * 128)

            nc.sync.dma_start(ident[:], idin[:])
            for c in range(CT):
                nc.sync.dma_start(kvs_sb[:, c, :], kvs[csl(c), :])
                nc.sync.dma_start(qcp_sb[:, c, :], qcp[csl(c), :])
                nc.sync.dma_start(
                    xT_sb[:, c, :, :],
                    xT[csl(c), :].rearrange("p (h w) -> p h w", w=SP))
            for c in range(CT):
                nc.sync.dma_start(wkvt_sb[:, c, :, :], wkvt[csl(c), :, :])
                nc.sync.dma_start(wqt_sb[:, c, :], wqt[csl(c), :])
                nc.sync.dma_start(kvb_sb[:, c, :], kvb[csl(c), :])
                nc.sync.dma_start(wlt_sb[:, c, :], wlt[csl(c), :])

            # diag stationaries for the K/V/Q convs, all on DVE (ACT is the
            # attention bottleneck; DVE is idle this early)
            dq_sb = cpool.tile([128, 9 * CT, 128], dt.bfloat16)
            for c in range(CT):
                for k in range(9):
                    nc.vector.tensor_scalar(
                        out=dk_sb[:, k * CT + c, :], in0=ident[:],
                        scalar1=kvs_sb[:, c, k:k + 1], scalar2=0.0,
                        op0=ALU.mult, op1=ALU.add)
                    nc.vector.tensor_scalar(
                        out=dv_sb[:, k * CT + c, :], in0=ident[:],
                        scalar1=kvs_sb[:, c, 9 + k:10 + k], scalar2=0.0,
                        op0=ALU.mult, op1=ALU.add)
                    nc.vector.tensor_scalar(
                        out=dq_sb[:, k * CT + c, :], in0=ident[:],
                        scalar1=qcp_sb[:, c, k:k + 1], scalar2=0.0,
                        op0=ALU.mult, op1=ALU.add)

            # persistent activations
            q_feat = cpool.tile([128, CT, T], dt.bfloat16)
            qh_sb = cpool.tile([128, CT, T], dt.bfloat16)
            kh_sb = cpool.tile([128, CT, T2], dt.bfloat16)
            vh_sb = cpool.tile([128, CT, T2], dt.bfloat16)
            vhT_sb = cpool.tile([128, NTT, HEADS * 65], dt.bfloat16)
            o_sb = cpool.tile([128, CT, T], dt.bfloat16)
            den_scr = cpool.tile([128, 2 * W], dt.float32)
            den_fold = cpool.tile([128, 2 * W // 8], dt.float32)
            r_fold = cpool.tile([128, 2 * W // 8], dt.float32)

            # ---- Phase 1: Q depthwise conv + BN on PE as diagonal-
            # stationary matmuls, 8 h-rows per chunk; BN bias at the DVE
            # evacuation. Rows 0:24 run pre-attention; the rest interleave
            # into the band loop as units.
            QROWS = 8

            def qconv_chunk(c, r0):
                x3 = xT_sb[:, c, :, :]  # [128, 58, 58] zero-padded
                ps = psA.tile([128, 2 * W], dt.float32, tag="psA")
                for k in range(9):
                    dy, dx = TAPS[k]
                    nc.tensor.matmul(
                        ps[:, 0:QROWS * S],
                        dq_sb[:, k * CT + c, :],
                        x3[:, 1 + dy + r0:1 + dy + r0 + QROWS,
                           1 + dx:1 + dx + S],
                        start=(k == 0), stop=(k == 8))
                nc.vector.tensor_scalar_add(
                    q_feat[:, c, r0 * S:(r0 + QROWS) * S],
                    ps[:, 0:QROWS * S],
                    qcp_sb[:, c, 9:10])

            # ------------- Phase 2: K/V stride-2 conv + projection ----------
            def kv_conv(d_sb, f_sb, bias_col):
                for c in range(CT):
                    x5 = xT_sb[:, c, :, :].rearrange(
                        "p (h sy) (w sx) -> p h sy w sx", sy=2, sx=2)
                    for ha, hb in ((0, 14), (14, 28)):
                        ps = psA.tile([128, 2 * W], dt.float32, tag="psA")
                        for k in range(9):
                            dy, dx = TAPS[k]
                            hoff, sy = ((0, 0) if dy == -1 else
                                        (0, 1) if dy == 0 else (1, 0))
                            woff, sx = ((0, 0) if dx == -1 else
                                        (0, 1) if dx == 0 else (1, 0))
                            nc.tensor.matmul(
                                ps[:, 0:(hb - ha) * S2],
                                d_sb[:, k * CT + c, :],
                                x5[:, ha + hoff:hb + hoff, sy,
                                   woff:woff + S2, sx],
                                start=(k == 0), stop=(k == 8))
                        nc.vector.tensor_scalar_add(
                            f_sb[:, c, ha * S2:hb * S2],
                            ps[:, 0:14 * S2],
                            kvb_sb[:, c, bias_col:bias_col + 1])

            def kv_proj(f_sb, dst_sb, wcol):
                for ot in range(CT):
                    osl = slice(ot * 128, (ot + 1) * 128)
                    for ha, hb in ((0, 14), (14, 28)):
                        ps = psA.tile([128, 2 * W], dt.float32, tag="psA")
                        for c in range(CT):
                            nc.tensor.matmul(
                                ps[:, 0:(hb - ha) * S2],
                                wkvt_sb[:, c, wcol, osl],
                                f_sb[:, c, ha * S2:hb * S2],
                                start=(c == 0), stop=(c == CT - 1))
                        nc.vector.tensor_copy(
                            dst_sb[:, ot, ha * S2:hb * S2],
                            ps[:, 0:14 * S2])

            kv_conv(dk_sb, kf_sb, 0)
            kv_proj(kf_sb, kh_sb, 0)
            kv_conv(dv_sb, vf_sb, 1)
            kv_proj(vf_sb, vh_sb, 1)

            # -------- Phase 3: vh^T [t, (head, 65)] with ones columns -------
            v4 = vhT_sb[:].rearrange("p n (h c) -> p n h c", c=65)
            nc.vector.memset(vhT_sb[:], 1.0)
            for tt in range(NTT):
                tsz = min(128, T2 - tt * 128)
                for ot in range(CT):
                    pst = psO.tile([128, 2 * W], dt.bfloat16, tag="psO")
                    nc.tensor.transpose(
                        pst[0:tsz, 0:128],
                        vh_sb[:, ot, tt * 128:tt * 128 + tsz],
                        ident[:])
                    nc.vector.tensor_copy(
                        v4[0:tsz, tt, 2 * ot:2 * ot + 2, 0:64],
                        pst[0:tsz, 0:128].rearrange("p (h c) -> p h c", c=64))

            # ---------------- Phase 4: Q projection chunks ------------------
            def qproj_unit(lc, ot):
                lsl = slice(lc * LCH, (lc + 1) * LCH)
                osl = slice(ot * 128, (ot + 1) * 128)
                ps = psA.tile([128, 2 * W], dt.float32, tag="psA")
                for c in range(CT):
                    nc.tensor.matmul(
                        ps[:, 0:LCH], wqt_sb[:, c, osl], q_feat[:, c, lsl],
                        start=(c == 0), stop=(c == CT - 1))
                nc.vector.tensor_copy(qh_sb[:, ot, lsl], ps[:, 0:LCH])

            for r0 in (0, 8, 16):
                for c in range(CT):
                    qconv_chunk(c, r0)
            for lc in (0, 1, 2):
                for ot in range(CT):
                    qproj_unit(lc, ot)

            # ---------------- Phase 5: attention ----------------
            def oproj_tile(lpos, lsz):
                ps = psA.tile([128, 2 * W], dt.float32, tag="psA")
                for c in range(CT):
                    nc.tensor.matmul(
                        ps[0:lsz, 0:DIM], o_sb[:, c, lpos:lpos + lsz],
                        wlt_sb[:, c, :],
                        start=(c == 0), stop=(c == CT - 1))
                ostage = opool.tile([128, DIM], dt.float32, tag="ostage")
                nc.vector.tensor_copy(ostage[0:lsz, :], ps[0:lsz, 0:DIM])
                nc.sync.dma_start(out[lpos:lpos + lsz, :], ostage[0:lsz, :])

            def norm_chain(ps_o, WW):
                """den row -> fold via DRAM -> recip -> broadcast [64, WW]."""
                fw = 8
                fp = WW // fw
                den_dr = dpool.tile([2 * W], dt.float32, tag="dd")
                r_dr = dpool.tile([2 * W], dt.float32, tag="rd")
                nc.vector.tensor_copy(den_scr[64:65, 0:WW],
                                      ps_o[64:65, 0:WW])
                nc.sync.dma_start(den_dr[None, 0:WW], den_scr[64:65, 0:WW])
                nc.sync.dma_start(
                    den_fold[0:fp, 0:fw],
                    den_dr[0:WW].rearrange("(p f) -> p f", f=fw))
                nc.vector.reciprocal(r_fold[0:fp, 0:fw],
                                     den_fold[0:fp, 0:fw])
                nc.sync.dma_start(
                    r_dr[0:WW].rearrange("(p f) -> p f", f=fw),
                    r_fold[0:fp, 0:fw])
                r_rep = rpool.tile([64, 2 * W], dt.float32, tag="r_rep")
                nc.sync.dma_start(r_rep[0:64, 0:WW],
                                  r_dr[None, 0:WW].to_broadcast([64, WW]))
                return r_rep

            # interleave queue: (kind, args) units issued between pairs
            def run_units(units, n):
                for _ in range(n):
                    if not units:
                        return
                    kind, args = units.pop(0)
                    if kind == 'oproj':
                        oproj_tile(*args)
                    elif kind == 'qconv':
                        qconv_chunk(*args)
                    else:
                        qproj_unit(*args)

            units = []
            for bi in range(NB):
                qs = bi * W
                # stage: conv rows for q-proj chunk lc=bi+3, that q-proj
                # chunk itself one band later, previous band's out-proj
                if bi < 4:
                    units += [('qconv', (c, 24 + 8 * bi)) for c in range(CT)]
                if 1 <= bi <= 4:
                    units += [('qproj', (bi + 2, ot)) for ot in range(CT)]
                if bi > 0:
                    pq = (bi - 1) * W
                    units += [('oproj', (pq + i * 128, 128)) for i in range(4)]

                for p in range(CT):  # head pair (2p, 2p+1), ctile p
                    hA, hB = 2 * p, 2 * p + 1
                    ps_o = psO.tile([128, 2 * W], dt.float32, tag="psO")

                    def scores_pair(tt):
                        tsz = min(128, T2 - tt * 128)
                        tslc = slice(tt * 128, tt * 128 + tsz)
                        ps = psA.tile([128, 2 * W], dt.float32, tag="psA")
                        nc.tensor.matmul(
                            ps[0:tsz, 0:W],
                            kh_sb[0:64, p, tslc],
                            qh_sb[0:64, p, qs:qs + W],
                            start=True, stop=True)
                        nc.tensor.matmul(
                            ps[0:tsz, W:2 * W],
                            kh_sb[64:128, p, tslc],
                            qh_sb[64:128, p, qs:qs + W],
                            start=True, stop=True)
                        return ps

                    # software pipeline: scores(tt+1) issues BEFORE the
                    # o-accumulation of tt, so the PE queue never holds a
                    # waiting o-matmul in front of ready scores work and
                    # ACT exp runs back-to-back.
                    ps_s = scores_pair(0)
                    for tt in range(NTT):
                        tsz = min(128, T2 - tt * 128)
                        e = epool.tile([128, 2 * W], dt.bfloat16, tag="e")
                        nc.scalar.activation(e[0:tsz, 0:2 * W],
                                             ps_s[0:tsz, 0:2 * W], AF.Exp)
                        if tt + 1 < NTT:
                            ps_s = scores_pair(tt + 1)
                        nc.tensor.matmul(
                            ps_o[0:65, 0:W],
                            vhT_sb[0:tsz, tt, hA * 65:hA * 65 + 65],
                            e[0:tsz, 0:W],
                            start=(tt == 0), stop=(tt == NTT - 1))
                        nc.tensor.matmul(
                            ps_o[0:65, W:2 * W],
                            vhT_sb[0:tsz, tt, hB * 65:hB * 65 + 65],
                            e[0:tsz, W:2 * W],
                            start=(tt == 0), stop=(tt == NTT - 1))
                    r_rep = norm_chain(ps_o, 2 * W)
                    nc.vector.tensor_tensor(
                        out=o_sb[0:64, p, qs:qs + W],
                        in0=ps_o[0:64, 0:W],
                        in1=r_rep[0:64, 0:W], op=ALU.mult)
                    nc.vector.tensor_tensor(
                        out=o_sb[64:128, p, qs:qs + W],
                        in0=ps_o[0:64, W:2 * W],
                        in1=r_rep[0:64, W:2 * W], op=ALU.mult)
                    # 2/3/4 units after pairs 0/1/2; every unit's inputs are
                    # at least one pair old, so PE never queues a waiting
                    # matmul ahead of live scores work.
                    run_units(units, 2 + p)

            # ---------------- Phase 6: 64-wide tail band ----------------
            # t-grouped scores: per pair ONE [tsz, 896] psum tile (cols
            # tt*128 + {0:64 A, 64:128 B}) -> ONE exp; shared ps_o
            # [65, 6*64] with one norm chain for all heads.
            qs, WT = NB * W, T - NB * W  # 3072, 64
            ps_o = psO.tile([128, 2 * W], dt.float32, tag="psO")
            for p in range(CT):
                hA, hB = 2 * p, 2 * p + 1
                for hh, h in ((0, hA), (1, hB)):
                    hsl = slice(64 * hh, 64 * hh + 64)
                    ps_s = psA.tile([128, 2 * W], dt.float32, tag="psA")
                    for tt in range(NTT):
                        tsz = min(128, T2 - tt * 128)
                        tslc = slice(tt * 128, tt * 128 + tsz)
                        nc.tensor.matmul(
                            ps_s[0:tsz, tt * 128:tt * 128 + WT],
                            kh_sb[hsl, p, tslc],
                            qh_sb[hsl, p, qs:qs + WT],
                            start=True, stop=True)
                    e = epool.tile([128, 2 * W], dt.bfloat16, tag="e")
                    nc.scalar.activation(e[0:128, 0:(NTT - 1) * 128],
                                         ps_s[0:128, 0:(NTT - 1) * 128],
                                         AF.Exp)
                    nc.scalar.activation(
                        e[0:16, (NTT - 1) * 128:NTT * 128],
                        ps_s[0:16, (NTT - 1) * 128:NTT * 128], AF.Exp)
                    # six head groups share one PSUM zero region (2KB bank)
                    # so each accumulation group fully closes before the
                    # next starts
                    for tt in range(NTT):
                        tsz = min(128, T2 - tt * 128)
                        nc.tensor.matmul(
                            ps_o[0:65, h * WT:h * WT + WT],
                            vhT_sb[0:tsz, tt, h * 65:h * 65 + 65],
                            e[0:tsz, tt * 128:tt * 128 + WT],
                            start=(tt == 0), stop=(tt == NTT - 1))
                # last 512-band's out-projection rides the tail band
                pq = (NB - 1) * W
                if p == 1:
                    oproj_tile(pq, 128)
                    oproj_tile(pq + 128, 128)
                elif p == 2:
                    oproj_tile(pq + 256, 128)
                    oproj_tile(pq + 384, 128)
            r_rep = norm_chain(ps_o, HEADS * WT)
            for h in range(HEADS):
                nc.vector.tensor_tensor(
                    out=o_sb[64 * (h % 2):64 * (h % 2) + 64, h // 2,
                             qs:qs + WT],
                    in0=ps_o[0:64, h * WT:h * WT + WT],
                    in1=r_rep[0:64, h * WT:h * WT + WT], op=ALU.mult)
            run_units(units, 99)
            oproj_tile(qs, WT)

    nc.compile()
    return nc


_CACHE = {}


def _prep_weights(inputs):
    import ml_dtypes
    bf16 = ml_dtypes.bfloat16
    f32 = np.float32

    def bn_fold(prefix):
        a = (np.asarray(inputs[f'bn{prefix}_s'], f32)
             / np.sqrt(np.asarray(inputs[f'bn{prefix}_v'], f32) + EPS))
        b = (np.asarray(inputs[f'bn{prefix}_b'], f32)
             - np.asarray(inputs[f'bn{prefix}_m'], f32) * a)
        return a.astype(f32), b.astype(f32)

    aq, bq = bn_fold('q')
    ak, bk = bn_fold('k')
    av, bv = bn_fold('v')

    conv_q = np.asarray(inputs['conv_q'], f32)[:, 0].reshape(DIM, 9)
    conv_k = np.asarray(inputs['conv_k'], f32)[:, 0].reshape(DIM, 9)
    conv_v = np.asarray(inputs['conv_v'], f32)[:, 0].reshape(DIM, 9)
    wq = np.asarray(inputs['wq'], f32)
    wk = np.asarray(inputs['wk'], f32)
    wv = np.asarray(inputs['wv'], f32)
    wl = np.asarray(inputs['w_last'], f32)

    qcp = np.zeros((DIM, 10), f32)
    qcp[:, :9] = conv_q * aq[:, None]
    qcp[:, 9] = bq

    wqt = np.ascontiguousarray((wq * SCALE).T).astype(bf16)  # [c, o]

    wkvt = np.stack([wk.T, wv.T], axis=1).astype(bf16)  # [c, {k,v}, o]
    kvs = np.concatenate([conv_k * ak[:, None], conv_v * av[:, None]],
                         axis=1).astype(f32)            # [c, 18]
    kvb = np.stack([bk, bv], axis=1).astype(f32)        # [c, 2]
    wlt = np.ascontiguousarray(wl.T).astype(bf16)
    idin = np.eye(128, dtype=bf16)
    return qcp, wqt, wkvt, kvs, kvb, wlt, idin


def _prep_x(xb):
    """[T, C] f32 -> zero-padded transposed [C, 58*58] bf16."""
    import ml_dtypes
    pad = np.zeros((DIM, S + 2, S + 2), np.float32)
    pad[:, 1:1 + S, 1:1 + S] = xb.T.reshape(DIM, S, S)
    return pad.reshape(DIM, (S + 2) * (S + 2)).astype(ml_dtypes.bfloat16)


def kernel(**inputs):
    from concourse.bass_utils import run_bass_kernel_spmd

    if 'nc' not in _CACHE:
        _CACHE['nc'] = build_program()
    nc = _CACHE['nc']

    qcp, wqt, wkvt, kvs, kvb, wlt, idin = _prep_weights(inputs)
    x = np.asarray(inputs['x'], np.float32)  # [8, T, C]
    B = x.shape[0]

    in_maps = []
    for b in range(B):
        in_maps.append({
            'xT': _prep_x(x[b]), 'qcp': qcp, 'wqt': wqt, 'wkvt': wkvt,
            'kvs': kvs, 'kvb': kvb, 'wlt': wlt, 'idin': idin,
        })

    res = run_bass_kernel_spmd(nc, in_maps, list(range(NCORES)))
    outs = np.stack([np.asarray(res.results[b]['out']) for b in range(B)],
                    axis=0)
    outs = outs + np.asarray(inputs['b_last'], np.float32)[None, None, :]
    return outs.astype(np.float32)


# revision 27
# speedup vs baseline: 1.2257x; 1.1938x over previous
"""Trainium2 Bass kernel for nn_AttentionConv (dense_transformer).

Sharding: data-parallel over batch — 8 NeuronCores, one batch image each.

Per-core dataflow (T=3136 tokens = 56x56, C=384, 6 heads x 64):
  - x shipped pre-transposed from host as xT [C, 58*58] bf16 (zero-padded).
  - Q path: depthwise 3x3 conv + BN tap-accumulated on DVE (first row-slab,
    so band 0 is ready early) + GPSIMD (remaining slabs, an otherwise-idle
    engine) via scalar_tensor_tensor; BN affine folded into tap scales and
    bias on host. Q projection on PE (softmax scale folded into wq).
  - K/V path: stride-2 depthwise conv as 9 diagonal-stationary PE matmuls
    (per-channel tap scale on the diagonal), BN bias at PSUM evacuation,
    then a plain [C->C] projection on PE.
  - Attention in 512-wide q bands, heads processed in PAIRS (2p, 2p+1 of
    the same 64-partition halves of ctile p):
      scores^T [t, q] for both heads land in ONE [128, 1024] f32 PSUM tile
      as two row-tiled matmuls (K=64 each, rows 0:64 / 64:128 of the PE
      array -> concurrent execution, ~2x scores throughput);
      ONE exp [tsz, 1024] on ACT covers the pair (ACT is the attention
      bottleneck; half the instruction count vs per-head exp);
      o^T accumulated per head via [vh | ones] stationaries into a shared
      [65, 1024] PSUM tile (A cols 0:512, B cols 512:1024) - the ones
      column yields both denominators on PSUM row 64, so the pair shares
      ONE norm chain (den row -> DRAM fold -> reciprocal -> broadcast).
  - Output projection + next-band Q projection interleaved into the band
    loop to fill PE while ACT runs exp. Tiny 64-wide tail band runs last
    (t-grouped scores -> 3 exps, one shared norm chain) so the serial
    tail is short. b_last added on host.
"""
import sys

sys.path.insert(0, '/opt/trn_rl_repo')

import numpy as np

DIM = 384
HEADS = 6
D = 64
S = 56           # stride-1 spatial side
S2 = 28          # stride-2 spatial side
T = S * S        # 3136
T2 = S2 * S2     # 784
EPS = 1e-5
SCALE = DIM ** -0.5
NCORES = 8
CT = DIM // 128          # 3 channel tiles
NTT = (T2 + 127) // 128  # 7 kv t-tiles (last = 16 rows)
W = 512                  # attention q band width
NB = 6                   # full bands; tail band is 64 wide
LCH = 448                # q-projection chunk (8 h-rows)

TAPS = [(dy, dx) for dy in (-1, 0, 1) for dx in (-1, 0, 1)]  # k=(dy+1)*3+(dx+1)


def build_program():
    import concourse.mybir as mybir
    from concourse import bacc
    from concourse.tile import TileContext

    dt = mybir.dt
    AF = mybir.ActivationFunctionType
    ALU = mybir.AluOpType

    nc = bacc.Bacc()

    SP = S + 2
    xT = nc.dram_tensor("xT", [DIM, SP * SP], dt.bfloat16,
                        kind="ExternalInput")
    qcp = nc.dram_tensor("qcp", [DIM, 10], dt.float32, kind="ExternalInput")
    wqt = nc.dram_tensor("wqt", [DIM, DIM], dt.bfloat16, kind="ExternalInput")
    wkvt = nc.dram_tensor("wkvt", [DIM, 2, DIM], dt.bfloat16,
                          kind="ExternalInput")
    kvs = nc.dram_tensor("kvs", [DIM, 18], dt.float32, kind="ExternalInput")
    kvb = nc.dram_tensor("kvb", [DIM, 2], dt.float32, kind="ExternalInput")
    wlt = nc.dram_tensor("wlt", [DIM, DIM], dt.bfloat16, kind="ExternalInput")
    idin = nc.dram_tensor("idin", [128, 128], dt.bfloat16, kind="ExternalInput")
    out = nc.dram_tensor("out", [T, DIM], dt.float32, kind="ExternalOutput")

    with TileContext(nc) as tc:
        with (
            tc.tile_pool(name="const", bufs=1) as cpool,
            tc.tile_pool(name="ework", bufs=3) as epool,
            tc.tile_pool(name="rrep", bufs=2) as rpool,
            tc.tile_pool(name="ostg", bufs=2) as opool,
            tc.tile_pool(name="psA", bufs=2, space="PSUM") as psA,
            tc.tile_pool(name="psO", bufs=2, space="PSUM") as psO,
            tc.tile_pool(name="dram", bufs=2, space="DRAM") as dpool,
        ):
            # ---------------- Phase 0: loads ----------------
            xT_sb = cpool.tile([128, CT, SP, SP], dt.bfloat16)
            qcp_sb = cpool.tile([128, CT, 10], dt.float32)
            wqt_sb = cpool.tile([128, CT, DIM], dt.bfloat16)
            dk_sb = cpool.tile([128, 9 * CT, 128], dt.bfloat16)
            dv_sb = cpool.tile([128, 9 * CT, 128], dt.bfloat16)
            kf_sb = cpool.tile([128, CT, T2], dt.bfloat16)
            vf_sb = cpool.tile([128, CT, T2], dt.bfloat16)
            kvb_sb = cpool.tile([128, CT, 2], dt.float32)
            wlt_sb = cpool.tile([128, CT, DIM], dt.bfloat16)
            ident = cpool.tile([128, 128], dt.bfloat16)
            wkvt_sb = cpool.tile([128, CT, 2, DIM], dt.bfloat16)
            kvs_sb = cpool.tile([128, CT, 18], dt.float32)

            def csl(c):
                return slice(c * 128, (c + 1) * 128)

            nc.sync.dma_start(ident[:], idin[:])
            for c in range(CT):
                nc.sync.dma_start(kvs_sb[:, c, :], kvs[csl(c), :])
                nc.sync.dma_start(qcp_sb[:, c, :], qcp[csl(c), :])
                nc.sync.dma_start(
                    xT_sb[:, c, :, :],
                    xT[csl(c), :].rearrange("p (h w) -> p h w", w=SP))
            for c in range(CT):
                nc.sync.dma_start(wkvt_sb[:, c, :, :], wkvt[csl(c), :, :])
                nc.sync.dma_start(wqt_sb[:, c, :], wqt[csl(c), :])
                nc.sync.dma_start(kvb_sb[:, c, :], kvb[csl(c), :])
                nc.sync.dma_start(wlt_sb[:, c, :], wlt[csl(c), :])

            # diag stationaries for the K/V/Q convs, all on DVE (ACT is the
            # attention bottleneck; DVE is idle this early)
            dq_sb = cpool.tile([128, 9 * CT, 128], dt.bfloat16)
            for c in range(CT):
                for k in range(9):
                    nc.vector.tensor_scalar(
                        out=dk_sb[:, k * CT + c, :], in0=ident[:],
                        scalar1=kvs_sb[:, c, k:k + 1], scalar2=0.0,
                        op0=ALU.mult, op1=ALU.add)
                    nc.vector.tensor_scalar(
                        out=dv_sb[:, k * CT + c, :], in0=ident[:],
                        scalar1=kvs_sb[:, c, 9 + k:10 + k], scalar2=0.0,
                        op0=ALU.mult, op1=ALU.add)
                    nc.vector.tensor_scalar(
                        out=dq_sb[:, k * CT + c, :], in0=ident[:],
                        scalar1=qcp_sb[:, c, k:k + 1], scalar2=0.0,
                        op0=ALU.mult, op1=ALU.add)

            # persistent activations
            q_feat = cpool.tile([128, CT, T], dt.bfloat16)
            qh_sb = cpool.tile([128, CT, T], dt.bfloat16)
            kh_sb = cpool.tile([128, CT, T2], dt.bfloat16)
            vh_sb = cpool.tile([128, CT, T2], dt.bfloat16)
            vhT_sb = cpool.tile([128, NTT, HEADS * 65], dt.bfloat16)
            o_sb = cpool.tile([128, CT, T], dt.bfloat16)
            den_scr = cpool.tile([128, 2 * W], dt.float32)
            den_fold = cpool.tile([128, 2 * W // 8], dt.float32)
            r_fold = cpool.tile([128, 2 * W // 8], dt.float32)

            # ---- Phase 1: Q depthwise conv + BN on PE as diagonal-
            # stationary matmuls, 8 h-rows per chunk; BN bias at the DVE
            # evacuation. Rows 0:24 run pre-attention; the rest interleave
            # into the band loop as units.
            QROWS = 8

            def qconv_chunks(c, r0s):
                """one or two 8-row conv chunks of ctile c; two chunks run
                their accumulation chains interleaved over two psA slots"""
                x3 = xT_sb[:, c, :, :]  # [128, 58, 58] zero-padded
                pss = [psA.tile([128, 2 * W], dt.float32, tag="psA",
                                name=f"qc_ps{i}") for i in range(len(r0s))]
                for k in range(9):
                    dy, dx = TAPS[k]
                    for ps, r0 in zip(pss, r0s):
                        nc.tensor.matmul(
                            ps[:, 0:QROWS * S],
                            dq_sb[:, k * CT + c, :],
                            x3[:, 1 + dy + r0:1 + dy + r0 + QROWS,
                               1 + dx:1 + dx + S],
                            start=(k == 0), stop=(k == 8))
                for ps, r0 in zip(pss, r0s):
                    nc.vector.tensor_scalar_add(
                        q_feat[:, c, r0 * S:(r0 + QROWS) * S],
                        ps[:, 0:QROWS * S],
                        qcp_sb[:, c, 9:10])

            # ------------- Phase 2: K/V stride-2 conv + projection ----------
            # The two half-chunks of each ctile run their 9-tap accumulation
            # chains INTERLEAVED across the two psA slots, so consecutive
            # matmul drains hit different PSUM banks and overlap (a single
            # chain into one bank serializes at ~2x the cost).
            def kv_conv(d_sb, f_sb, bias_col):
                for c in range(CT):
                    x5 = xT_sb[:, c, :, :].rearrange(
                        "p (h sy) (w sx) -> p h sy w sx", sy=2, sx=2)
                    ps0 = psA.tile([128, 2 * W], dt.float32, tag="psA")
                    ps1 = psA.tile([128, 2 * W], dt.float32, tag="psA")
                    for k in range(9):
                        dy, dx = TAPS[k]
                        hoff, sy = ((0, 0) if dy == -1 else
                                    (0, 1) if dy == 0 else (1, 0))
                        woff, sx = ((0, 0) if dx == -1 else
                                    (0, 1) if dx == 0 else (1, 0))
                        for ps, (ha, hb) in ((ps0, (0, 14)), (ps1, (14, 28))):
                            nc.tensor.matmul(
                                ps[:, 0:(hb - ha) * S2],
                                d_sb[:, k * CT + c, :],
                                x5[:, ha + hoff:hb + hoff, sy,
                                   woff:woff + S2, sx],
                                start=(k == 0), stop=(k == 8))
                    for ps, (ha, hb) in ((ps0, (0, 14)), (ps1, (14, 28))):
                        nc.vector.tensor_scalar_add(
                            f_sb[:, c, ha * S2:hb * S2],
                            ps[:, 0:14 * S2],
                            kvb_sb[:, c, bias_col:bias_col + 1])

            def kv_proj(f_sb, dst_sb, wcol):
                for ot in range(CT):
                    osl = slice(ot * 128, (ot + 1) * 128)
                    for ha, hb in ((0, 14), (14, 28)):
                        ps = psA.tile([128, 2 * W], dt.float32, tag="psA")
                        for c in range(CT):
                            nc.tensor.matmul(
                                ps[:, 0:(hb - ha) * S2],
                                wkvt_sb[:, c, wcol, osl],
                                f_sb[:, c, ha * S2:hb * S2],
                                start=(c == 0), stop=(c == CT - 1))
                        nc.vector.tensor_copy(
                            dst_sb[:, ot, ha * S2:hb * S2],
                            ps[:, 0:14 * S2])

            kv_conv(dk_sb, kf_sb, 0)
            kv_proj(kf_sb, kh_sb, 0)
            kv_conv(dv_sb, vf_sb, 1)
            kv_proj(vf_sb, vh_sb, 1)

            # -------- Phase 3: vh^T [t, (head, 65)] with ones columns -------
            v4 = vhT_sb[:].rearrange("p n (h c) -> p n h c", c=65)
            nc.vector.memset(vhT_sb[:], 1.0)
            for tt in range(NTT):
                tsz = min(128, T2 - tt * 128)
                for ot in range(CT):
                    pst = psO.tile([128, 2 * W], dt.bfloat16, tag="psO")
                    nc.tensor.transpose(
                        pst[0:tsz, 0:128],
                        vh_sb[:, ot, tt * 128:tt * 128 + tsz],
                        ident[:])
                    nc.vector.tensor_copy(
                        v4[0:tsz, tt, 2 * ot:2 * ot + 2, 0:64],
                        pst[0:tsz, 0:128].rearrange("p (h c) -> p h c", c=64))

            # ---------------- Phase 4: Q projection chunks ------------------
            def qproj_unit(lc, ot):
                lsl = slice(lc * LCH, (lc + 1) * LCH)
                osl = slice(ot * 128, (ot + 1) * 128)
                ps = psA.tile([128, 2 * W], dt.float32, tag="psA")
                for c in range(CT):
                    nc.tensor.matmul(
                        ps[:, 0:LCH], wqt_sb[:, c, osl], q_feat[:, c, lsl],
                        start=(c == 0), stop=(c == CT - 1))
                nc.vector.tensor_copy(qh_sb[:, ot, lsl], ps[:, 0:LCH])

            for c in range(CT):
                qconv_chunks(c, (0, 8))
                qconv_chunks(c, (16, 24))
            for lc in (0, 1, 2):
                for ot in range(CT):
                    qproj_unit(lc, ot)

            # ---------------- Phase 5: attention ----------------
            def oproj_tile(lpos, lsz):
                ps = psA.tile([128, 2 * W], dt.float32, tag="psA")
                for c in range(CT):
                    nc.tensor.matmul(
                        ps[0:lsz, 0:DIM], o_sb[:, c, lpos:lpos + lsz],
                        wlt_sb[:, c, :],
                        start=(c == 0), stop=(c == CT - 1))
                ostage = opool.tile([128, DIM], dt.float32, tag="ostage")
                nc.vector.tensor_copy(ostage[0:lsz, :], ps[0:lsz, 0:DIM])
                nc.sync.dma_start(out[lpos:lpos + lsz, :], ostage[0:lsz, :])

            def norm_chain(ps_o, WW):
                """den row -> fold via DRAM -> recip -> broadcast [64, WW]."""
                fw = 8
                fp = WW // fw
                den_dr = dpool.tile([2 * W], dt.float32, tag="dd")
                r_dr = dpool.tile([2 * W], dt.float32, tag="rd")
                nc.vector.tensor_copy(den_scr[64:65, 0:WW],
                                      ps_o[64:65, 0:WW])
                nc.sync.dma_start(den_dr[None, 0:WW], den_scr[64:65, 0:WW])
                nc.sync.dma_start(
                    den_fold[0:fp, 0:fw],
                    den_dr[0:WW].rearrange("(p f) -> p f", f=fw))
                nc.vector.reciprocal(r_fold[0:fp, 0:fw],
                                     den_fold[0:fp, 0:fw])
                nc.sync.dma_start(
                    r_dr[0:WW].rearrange("(p f) -> p f", f=fw),
                    r_fold[0:fp, 0:fw])
                r_rep = rpool.tile([64, 2 * W], dt.float32, tag="r_rep")
                nc.sync.dma_start(r_rep[0:64, 0:WW],
                                  r_dr[None, 0:WW].to_broadcast([64, WW]))
                return r_rep

            # interleave queue: (kind, args) units issued between pairs
            def run_units(units, n):
                for _ in range(n):
                    if not units:
                        return
                    kind, args = units.pop(0)
                    if kind == 'oproj':
                        oproj_tile(*args)
                    elif kind == 'qconv':
                        qconv_chunks(*args)
                    else:
                        qproj_unit(*args)

            def scores_pair(qs, p, tt):
                tsz = min(128, T2 - tt * 128)
                tslc = slice(tt * 128, tt * 128 + tsz)
                ps = psA.tile([128, 2 * W], dt.float32, tag="psA")
                nc.tensor.matmul(
                    ps[0:tsz, 0:W],
                    kh_sb[0:64, p, tslc],
                    qh_sb[0:64, p, qs:qs + W],
                    start=True, stop=True)
                nc.tensor.matmul(
                    ps[0:tsz, W:2 * W],
                    kh_sb[64:128, p, tslc],
                    qh_sb[64:128, p, qs:qs + W],
                    start=True, stop=True)
                return ps

            # Flat cross-pair software pipeline: the NEXT pair's first
            # scores issue before this pair's last o-accumulation, so ACT
            # exp never has a pair-boundary hole. Units (qconv/qproj/oproj,
            # all with inputs >= one pair old) drip in mid-pair.
            units = []
            all_pairs = [(bi, p) for bi in range(NB) for p in range(CT)]
            ps_s = None
            for idx, (bi, p) in enumerate(all_pairs):
                qs = bi * W
                if p == 0:
                    # band start: stage conv rows for q-proj chunk lc=bi+3,
                    # that q-proj one band later, previous band's out-proj
                    # (added last so it is consumed late, after the previous
                    # band's final evacuation has certainly landed).
                    if bi < 3:
                        units += [('qconv', (c, (32 + 8 * bi,)))
                                  for c in range(CT)]
                    if 1 <= bi <= 4:
                        units += [('qproj', (bi + 2, ot)) for ot in range(CT)]
                    if bi > 0:
                        pq = (bi - 1) * W
                        units += [('oproj', (pq + i * 128, 128))
                                  for i in range(4)]

                hA, hB = 2 * p, 2 * p + 1
                ps_o = psO.tile([128, 2 * W], dt.float32, tag="psO")
                if idx == 0:
                    ps_s = scores_pair(qs, p, 0)
                for tt in range(NTT):
                    tsz = min(128, T2 - tt * 128)
                    e = epool.tile([128, 2 * W], dt.bfloat16, tag="e")
                    nc.scalar.activation(e[0:tsz, 0:2 * W],
                                         ps_s[0:tsz, 0:2 * W], AF.Exp)
                    if tt + 1 < NTT:
                        ps_s = scores_pair(qs, p, tt + 1)
                    elif idx + 1 < len(all_pairs):
                        nbi, np_ = all_pairs[idx + 1]
                        ps_s = scores_pair(nbi * W, np_, 0)
                    nc.tensor.matmul(
                        ps_o[0:65, 0:W],
                        vhT_sb[0:tsz, tt, hA * 65:hA * 65 + 65],
                        e[0:tsz, 0:W],
                        start=(tt == 0), stop=(tt == NTT - 1))
                    nc.tensor.matmul(
                        ps_o[0:65, W:2 * W],
                        vhT_sb[0:tsz, tt, hB * 65:hB * 65 + 65],
                        e[0:tsz, W:2 * W],
                        start=(tt == 0), stop=(tt == NTT - 1))
                    if tt in (1, 3):
                        run_units(units, 1)
                r_rep = norm_chain(ps_o, 2 * W)
                nc.vector.tensor_tensor(
                    out=o_sb[0:64, p, qs:qs + W],
                    in0=ps_o[0:64, 0:W],
                    in1=r_rep[0:64, 0:W], op=ALU.mult)
                nc.vector.tensor_tensor(
                    out=o_sb[64:128, p, qs:qs + W],
                    in0=ps_o[0:64, W:2 * W],
                    in1=r_rep[0:64, W:2 * W], op=ALU.mult)
                run_units(units, 1)

            # ---------------- Phase 6: 64-wide tail band ----------------
            # t-grouped scores: per pair ONE [tsz, 896] psum tile (cols
            # tt*128 + {0:64 A, 64:128 B}) -> ONE exp; shared ps_o
            # [65, 6*64] with one norm chain for all heads.
            qs, WT = NB * W, T - NB * W  # 3072, 64
            ps_o = psO.tile([128, 2 * W], dt.float32, tag="psO")
            for p in range(CT):
                hA, hB = 2 * p, 2 * p + 1
                for hh, h in ((0, hA), (1, hB)):
                    hsl = slice(64 * hh, 64 * hh + 64)
                    ps_s = psA.tile([128, 2 * W], dt.float32, tag="psA")
                    for tt in range(NTT):
                        tsz = min(128, T2 - tt * 128)
                        tslc = slice(tt * 128, tt * 128 + tsz)
                        nc.tensor.matmul(
                            ps_s[0:tsz, tt * 128:tt * 128 + WT],
                            kh_sb[hsl, p, tslc],
                            qh_sb[hsl, p, qs:qs + WT],
                            start=True, stop=True)
                    e = epool.tile([128, 2 * W], dt.bfloat16, tag="e")
                    nc.scalar.activation(e[0:128, 0:(NTT - 1) * 128],
                                         ps_s[0:128, 0:(NTT - 1) * 128],
                                         AF.Exp)
                    nc.scalar.activation(
                        e[0:16, (NTT - 1) * 128:NTT * 128],
                        ps_s[0:16, (NTT - 1) * 128:NTT * 128], AF.Exp)
                    # six head groups share one PSUM zero region (2KB bank)
                    # so each accumulation group fully closes before the
                    # next starts
                    for tt in range(NTT):
                        tsz = min(128, T2 - tt * 128)
                        nc.tensor.matmul(
                            ps_o[0:65, h * WT:h * WT + WT],
                            vhT_sb[0:tsz, tt, h * 65:h * 65 + 65],
                            e[0:tsz, tt * 128:tt * 128 + WT],
                            start=(tt == 0), stop=(tt == NTT - 1))
                # last 512-band's out-projection rides the tail band
                pq = (NB - 1) * W
                if p == 1:
                    oproj_tile(pq, 128)
                    oproj_tile(pq + 128, 128)
                elif p == 2:
                    oproj_tile(pq + 256, 128)
                    oproj_tile(pq + 384, 128)
            r_rep = norm_chain(ps_o, HEADS * WT)
            for h in range(HEADS):
                nc.vector.tensor_tensor(
                    out=o_sb[64 * (h % 2):64 * (h % 2) + 64, h // 2,
                             qs:qs + WT],
                    in0=ps_o[0:64, h * WT:h * WT + WT],
                    in1=r_rep[0:64, h * WT:h * WT + WT], op=ALU.mult)
            run_units(units, 99)
            oproj_tile(qs, WT)

    nc.compile()
    return nc


_CACHE = {}


def _prep_weights(inputs):
    import ml_dtypes
    bf16 = ml_dtypes.bfloat16
    f32 = np.float32

    def bn_fold(prefix):
        a = (np.asarray(inputs[f'bn{prefix}_s'], f32)
             / np.sqrt(np.asarray(inputs[f'bn{prefix}_v'], f32) + EPS))
        b = (np.asarray(inputs[f'bn{prefix}_b'], f32)
             - np.asarray(inputs[f'bn{prefix}_m'], f32) * a)
        return a.astype(f32), b.astype(f32)

    aq, bq = bn_fold('q')
    ak, bk = bn_fold('k')
    av, bv = bn_fold('v')

    conv_q = np.asarray(inputs['conv_q'], f32)[:, 0].reshape(DIM, 9)
    conv_k = np.asarray(inputs['conv_k'], f32)[:, 0].reshape(DIM, 9)
    conv_v = np.asarray(inputs['conv_v'], f32)[:, 0].reshape(DIM, 9)
    wq = np.asarray(inputs['wq'], f32)
    wk = np.asarray(inputs['wk'], f32)
    wv = np.asarray(inputs['wv'], f32)
    wl = np.asarray(inputs['w_last'], f32)

    qcp = np.zeros((DIM, 10), f32)
    qcp[:, :9] = conv_q * aq[:, None]
    qcp[:, 9] = bq

    wqt = np.ascontiguousarray((wq * SCALE).T).astype(bf16)  # [c, o]

    wkvt = np.stack([wk.T, wv.T], axis=1).astype(bf16)  # [c, {k,v}, o]
    kvs = np.concatenate([conv_k * ak[:, None], conv_v * av[:, None]],
                         axis=1).astype(f32)            # [c, 18]
    kvb = np.stack([bk, bv], axis=1).astype(f32)        # [c, 2]
    wlt = np.ascontiguousarray(wl.T).astype(bf16)
    idin = np.eye(128, dtype=bf16)
    return qcp, wqt, wkvt, kvs, kvb, wlt, idin


def _prep_x(xb):
    """[T, C] f32 -> zero-padded transposed [C, 58*58] bf16."""
    import ml_dtypes
    pad = np.zeros((DIM, S + 2, S + 2), np.float32)
    pad[:, 1:1 + S, 1:1 + S] = xb.T.reshape(DIM, S, S)
    return pad.reshape(DIM, (S + 2) * (S + 2)).astype(ml_dtypes.bfloat16)


def kernel(**inputs):
    from concourse.bass_utils import run_bass_kernel_spmd

    if 'nc' not in _CACHE:
        _CACHE['nc'] = build_program()
    nc = _CACHE['nc']

    qcp, wqt, wkvt, kvs, kvb, wlt, idin = _prep_weights(inputs)
    x = np.asarray(inputs['x'], np.float32)  # [8, T, C]
    B = x.shape[0]

    in_maps = []
    for b in range(B):
        in_maps.append({
            'xT': _prep_x(x[b]), 'qcp': qcp, 'wqt': wqt, 'wkvt': wkvt,
            'kvs': kvs, 'kvb': kvb, 'wlt': wlt, 'idin': idin,
        })

    res = run_bass_kernel_spmd(nc, in_maps, list(range(NCORES)))
    outs = np.stack([np.asarray(res.results[b]['out']) for b in range(B)],
                    axis=0)
    outs = outs + np.asarray(inputs['b_last'], np.float32)[None, None, :]
    return outs.astype(np.float32)


# revision 32
# speedup vs baseline: 1.2360x; 1.0084x over previous
"""Trainium2 Bass kernel for nn_AttentionConv (dense_transformer).

Sharding: data-parallel over batch — 8 NeuronCores, one batch image each.

Per-core dataflow (T=3136 tokens = 56x56, C=384, 6 heads x 64):
  - x shipped pre-transposed from host as xT [C, 58*58] bf16 (zero-padded).
  - Q path: depthwise 3x3 conv + BN tap-accumulated on DVE (first row-slab,
    so band 0 is ready early) + GPSIMD (remaining slabs, an otherwise-idle
    engine) via scalar_tensor_tensor; BN affine folded into tap scales and
    bias on host. Q projection on PE (softmax scale folded into wq).
  - K/V path: stride-2 depthwise conv as 9 diagonal-stationary PE matmuls
    (per-channel tap scale on the diagonal), BN bias at PSUM evacuation,
    then a plain [C->C] projection on PE.
  - Attention in 512-wide q bands, heads processed in PAIRS (2p, 2p+1 of
    the same 64-partition halves of ctile p):
      scores^T [t, q] for both heads land in ONE [128, 1024] f32 PSUM tile
      as two row-tiled matmuls (K=64 each, rows 0:64 / 64:128 of the PE
      array -> concurrent execution, ~2x scores throughput);
      ONE exp [tsz, 1024] on ACT covers the pair (ACT is the attention
      bottleneck; half the instruction count vs per-head exp);
      o^T accumulated per head via [vh | ones] stationaries into a shared
      [65, 1024] PSUM tile (A cols 0:512, B cols 512:1024) - the ones
      column yields both denominators on PSUM row 64, so the pair shares
      ONE norm chain (den row -> DRAM fold -> reciprocal -> broadcast).
  - Output projection + next-band Q projection interleaved into the band
    loop to fill PE while ACT runs exp. Tiny 64-wide tail band runs last
    (t-grouped scores -> 3 exps, one shared norm chain) so the serial
    tail is short. b_last added on host.
"""
import sys

sys.path.insert(0, '/opt/trn_rl_repo')

import numpy as np

DIM = 384
HEADS = 6
D = 64
S = 56           # stride-1 spatial side
S2 = 28          # stride-2 spatial side
T = S * S        # 3136
T2 = S2 * S2     # 784
EPS = 1e-5
SCALE = DIM ** -0.5
NCORES = 8
CT = DIM // 128          # 3 channel tiles
NTT = (T2 + 127) // 128  # 7 kv t-tiles (last = 16 rows)
W = 512                  # attention q band width
NB = 6                   # full bands; tail band is 64 wide
LCH = 448                # q-projection chunk (8 h-rows)

TAPS = [(dy, dx) for dy in (-1, 0, 1) for dx in (-1, 0, 1)]  # k=(dy+1)*3+(dx+1)


def build_program():
    import concourse.mybir as mybir
    from concourse import bacc
    from concourse.tile import TileContext

    dt = mybir.dt
    AF = mybir.ActivationFunctionType
    ALU = mybir.AluOpType

    nc = bacc.Bacc()

    SP = S + 2
    xT = nc.dram_tensor("xT", [DIM, SP * SP], dt.bfloat16,
                        kind="ExternalInput")
    qcp = nc.dram_tensor("qcp", [DIM, 10], dt.float32, kind="ExternalInput")
    wqt = nc.dram_tensor("wqt", [DIM, DIM], dt.bfloat16, kind="ExternalInput")
    wkvt = nc.dram_tensor("wkvt", [DIM, 2, DIM], dt.bfloat16,
                          kind="ExternalInput")
    kvs = nc.dram_tensor("kvs", [DIM, 18], dt.float32, kind="ExternalInput")
    kvb = nc.dram_tensor("kvb", [DIM, 2], dt.float32, kind="ExternalInput")
    wlt = nc.dram_tensor("wlt", [DIM, DIM], dt.bfloat16, kind="ExternalInput")
    idin = nc.dram_tensor("idin", [128, 128], dt.bfloat16, kind="ExternalInput")
    out = nc.dram_tensor("out", [T, DIM], dt.float32, kind="ExternalOutput")

    with TileContext(nc) as tc:
        with (
            tc.tile_pool(name="const", bufs=1) as cpool,
            tc.tile_pool(name="ework", bufs=3) as epool,
            tc.tile_pool(name="rrep", bufs=2) as rpool,
            tc.tile_pool(name="ostg", bufs=2) as opool,
            tc.tile_pool(name="psA", bufs=2, space="PSUM") as psA,
            tc.tile_pool(name="psO", bufs=2, space="PSUM") as psO,
            tc.tile_pool(name="dram", bufs=2, space="DRAM") as dpool,
        ):
            # ---------------- Phase 0: loads ----------------
            xT_sb = cpool.tile([128, CT, SP, SP], dt.bfloat16)
            qcp_sb = cpool.tile([128, CT, 10], dt.float32)
            wqt_sb = cpool.tile([128, CT, DIM], dt.bfloat16)
            dk_sb = cpool.tile([128, 9 * CT, 128], dt.bfloat16)
            dv_sb = cpool.tile([128, 9 * CT, 128], dt.bfloat16)
            kf_sb = cpool.tile([128, CT, T2], dt.bfloat16)
            vf_sb = cpool.tile([128, CT, T2], dt.bfloat16)
            kvb_sb = cpool.tile([128, CT, 2], dt.float32)
            wlt_sb = cpool.tile([128, CT, DIM], dt.bfloat16)
            ident = cpool.tile([128, 128], dt.bfloat16)
            wkvt_sb = cpool.tile([128, CT, 2, DIM], dt.bfloat16)
            kvs_sb = cpool.tile([128, CT, 18], dt.float32)

            def csl(c):
                return slice(c * 128, (c + 1) * 128)

            nc.sync.dma_start(ident[:], idin[:])
            for c in range(CT):
                nc.sync.dma_start(kvs_sb[:, c, :], kvs[csl(c), :])
                nc.sync.dma_start(qcp_sb[:, c, :], qcp[csl(c), :])
                nc.sync.dma_start(
                    xT_sb[:, c, :, :],
                    xT[csl(c), :].rearrange("p (h w) -> p h w", w=SP))
            for c in range(CT):
                nc.sync.dma_start(wkvt_sb[:, c, :, :], wkvt[csl(c), :, :])
                nc.sync.dma_start(wqt_sb[:, c, :], wqt[csl(c), :])
                nc.sync.dma_start(kvb_sb[:, c, :], kvb[csl(c), :])
                nc.sync.dma_start(wlt_sb[:, c, :], wlt[csl(c), :])

            # diag stationaries: dk on DVE (gates K conv, earliest), dv/dq
            # on ACT (idle until attention)
            dq_sb = cpool.tile([128, 9 * CT, 128], dt.bfloat16)
            for c in range(CT):
                for k in range(9):
                    nc.vector.tensor_scalar(
                        out=dk_sb[:, k * CT + c, :], in0=ident[:],
                        scalar1=kvs_sb[:, c, k:k + 1], scalar2=0.0,
                        op0=ALU.mult, op1=ALU.add)
                    nc.scalar.activation(
                        dv_sb[:, k * CT + c, :], ident[:],
                        AF.Copy, scale=kvs_sb[:, c, 9 + k:10 + k])
                    nc.scalar.activation(
                        dq_sb[:, k * CT + c, :], ident[:],
                        AF.Copy, scale=qcp_sb[:, c, k:k + 1])

            # persistent activations
            q_feat = cpool.tile([128, CT, T], dt.bfloat16)
            qh_sb = cpool.tile([128, CT, T], dt.bfloat16)
            kh_sb = cpool.tile([128, CT, T2], dt.bfloat16)
            vh_sb = cpool.tile([128, CT, T2], dt.bfloat16)
            vhT_sb = cpool.tile([128, NTT, HEADS * 65], dt.bfloat16)
            o_sb = cpool.tile([128, CT, T], dt.bfloat16)
            den_scr = cpool.tile([128, 2 * W], dt.float32)
            den_fold = cpool.tile([128, 2 * W // 8], dt.float32)
            r_fold = cpool.tile([128, 2 * W // 8], dt.float32)

            # ---- Phase 1: Q depthwise conv + BN on PE as diagonal-
            # stationary matmuls, 8 h-rows per chunk; BN bias at the DVE
            # evacuation. Rows 0:24 run pre-attention; the rest interleave
            # into the band loop as units.
            QROWS = 8

            def qconv_chunks(c, r0s):
                """one or two 8-row conv chunks of ctile c; two chunks run
                their accumulation chains interleaved over two psA slots"""
                x3 = xT_sb[:, c, :, :]  # [128, 58, 58] zero-padded
                pss = [psA.tile([128, 2 * W], dt.float32, tag="psA",
                                name=f"qc_ps{i}") for i in range(len(r0s))]
                for k in range(9):
                    dy, dx = TAPS[k]
                    for ps, r0 in zip(pss, r0s):
                        nc.tensor.matmul(
                            ps[:, 0:QROWS * S],
                            dq_sb[:, k * CT + c, :],
                            x3[:, 1 + dy + r0:1 + dy + r0 + QROWS,
                               1 + dx:1 + dx + S],
                            start=(k == 0), stop=(k == 8))
                for ps, r0 in zip(pss, r0s):
                    nc.vector.tensor_scalar_add(
                        q_feat[:, c, r0 * S:(r0 + QROWS) * S],
                        ps[:, 0:QROWS * S],
                        qcp_sb[:, c, 9:10])

            # ------------- Phase 2: K/V stride-2 conv + projection ----------
            # The two half-chunks of each ctile run their 9-tap accumulation
            # chains INTERLEAVED across the two psA slots, so consecutive
            # matmul drains hit different PSUM banks and overlap (a single
            # chain into one bank serializes at ~2x the cost).
            def kv_conv(d_sb, f_sb, bias_col):
                for c in range(CT):
                    x5 = xT_sb[:, c, :, :].rearrange(
                        "p (h sy) (w sx) -> p h sy w sx", sy=2, sx=2)
                    ps0 = psA.tile([128, 2 * W], dt.float32, tag="psA")
                    ps1 = psA.tile([128, 2 * W], dt.float32, tag="psA")
                    for k in range(9):
                        dy, dx = TAPS[k]
                        hoff, sy = ((0, 0) if dy == -1 else
                                    (0, 1) if dy == 0 else (1, 0))
                        woff, sx = ((0, 0) if dx == -1 else
                                    (0, 1) if dx == 0 else (1, 0))
                        for ps, (ha, hb) in ((ps0, (0, 14)), (ps1, (14, 28))):
                            nc.tensor.matmul(
                                ps[:, 0:(hb - ha) * S2],
                                d_sb[:, k * CT + c, :],
                                x5[:, ha + hoff:hb + hoff, sy,
                                   woff:woff + S2, sx],
                                start=(k == 0), stop=(k == 8))
                    for ps, (ha, hb) in ((ps0, (0, 14)), (ps1, (14, 28))):
                        nc.vector.tensor_scalar_add(
                            f_sb[:, c, ha * S2:hb * S2],
                            ps[:, 0:14 * S2],
                            kvb_sb[:, c, bias_col:bias_col + 1])

            def kv_proj(f_sb, dst_sb, wcol):
                for ot in range(CT):
                    osl = slice(ot * 128, (ot + 1) * 128)
                    for ha, hb in ((0, 14), (14, 28)):
                        ps = psA.tile([128, 2 * W], dt.float32, tag="psA")
                        for c in range(CT):
                            nc.tensor.matmul(
                                ps[:, 0:(hb - ha) * S2],
                                wkvt_sb[:, c, wcol, osl],
                                f_sb[:, c, ha * S2:hb * S2],
                                start=(c == 0), stop=(c == CT - 1))
                        nc.vector.tensor_copy(
                            dst_sb[:, ot, ha * S2:hb * S2],
                            ps[:, 0:14 * S2])

            # Q conv tap-accumulation on DVE, in place in q_feat bf16:
            # out = x_tap * scale (+ bias, k=0) [+ acc]
            def qconv_dve(c, r0, r1, ks):
                x3 = xT_sb[:, c, :, :]
                dst = q_feat[:, c, r0 * S:r1 * S].rearrange(
                    "p (h w) -> p h w", w=S)
                for k in ks:
                    dy, dx = TAPS[k]
                    xs = x3[:, 1 + dy + r0:1 + dy + r1, 1 + dx:1 + dx + S]
                    if k == 0:
                        nc.vector.tensor_scalar(
                            out=dst, in0=xs, scalar1=qcp_sb[:, c, 0:1],
                            scalar2=qcp_sb[:, c, 9:10],
                            op0=ALU.mult, op1=ALU.add)
                    else:
                        nc.vector.scalar_tensor_tensor(
                            out=dst, in0=xs, scalar=qcp_sb[:, c, k:k + 1],
                            in1=dst, op0=ALU.mult, op1=ALU.add)

            # rows 0:16 in 3-tap subgroups dripped between KV-phase stages
            # (keeps the KV evac/copy stream on DVE from queueing behind a
            # long conv batch)
            qdve_pre = [(c, ks) for c in range(CT)
                        for ks in ((0, 1, 2), (3, 4, 5), (6, 7, 8))]

            def drip_qdve(n):
                for _ in range(n):
                    if qdve_pre:
                        c, ks = qdve_pre.pop(0)
                        qconv_dve(c, 0, 16, ks)

            kv_conv(dk_sb, kf_sb, 0)
            drip_qdve(2)
            kv_proj(kf_sb, kh_sb, 0)
            drip_qdve(2)
            kv_conv(dv_sb, vf_sb, 1)
            drip_qdve(2)
            kv_proj(vf_sb, vh_sb, 1)
            drip_qdve(2)

            # -------- Phase 3: vh^T [t, (head, 65)] with ones columns -------
            v4 = vhT_sb[:].rearrange("p n (h c) -> p n h c", c=65)
            nc.vector.memset(vhT_sb[:], 1.0)
            for tt in range(NTT):
                tsz = min(128, T2 - tt * 128)
                for ot in range(CT):
                    pst = psO.tile([128, 2 * W], dt.bfloat16, tag="psO")
                    nc.tensor.transpose(
                        pst[0:tsz, 0:128],
                        vh_sb[:, ot, tt * 128:tt * 128 + tsz],
                        ident[:])
                    nc.vector.tensor_copy(
                        v4[0:tsz, tt, 2 * ot:2 * ot + 2, 0:64],
                        pst[0:tsz, 0:128].rearrange("p (h c) -> p h c", c=64))

            # ---------------- Phase 4: Q projection chunks ------------------
            def qproj_unit(lc, ot):
                lsl = slice(lc * LCH, (lc + 1) * LCH)
                osl = slice(ot * 128, (ot + 1) * 128)
                ps = psA.tile([128, 2 * W], dt.float32, tag="psA")
                for c in range(CT):
                    nc.tensor.matmul(
                        ps[:, 0:LCH], wqt_sb[:, c, osl], q_feat[:, c, lsl],
                        start=(c == 0), stop=(c == CT - 1))
                nc.vector.tensor_copy(qh_sb[:, ot, lsl], ps[:, 0:LCH])

            drip_qdve(9)
            for c in range(CT):
                qconv_chunks(c, (16, 24))
            for lc in (0, 1, 2):
                for ot in range(CT):
                    qproj_unit(lc, ot)

            # ---------------- Phase 5: attention ----------------
            def oproj_tile(lpos, lsz):
                ps = psA.tile([128, 2 * W], dt.float32, tag="psA")
                for c in range(CT):
                    nc.tensor.matmul(
                        ps[0:lsz, 0:DIM], o_sb[:, c, lpos:lpos + lsz],
                        wlt_sb[:, c, :],
                        start=(c == 0), stop=(c == CT - 1))
                ostage = opool.tile([128, DIM], dt.float32, tag="ostage")
                nc.vector.tensor_copy(ostage[0:lsz, :], ps[0:lsz, 0:DIM])
                nc.sync.dma_start(out[lpos:lpos + lsz, :], ostage[0:lsz, :])

            def norm_chain(ps_o, WW):
                """den row -> fold via DRAM -> recip -> broadcast [64, WW]."""
                fw = 8
                fp = WW // fw
                den_dr = dpool.tile([2 * W], dt.float32, tag="dd")
                r_dr = dpool.tile([2 * W], dt.float32, tag="rd")
                nc.vector.tensor_copy(den_scr[64:65, 0:WW],
                                      ps_o[64:65, 0:WW])
                nc.sync.dma_start(den_dr[None, 0:WW], den_scr[64:65, 0:WW])
                nc.sync.dma_start(
                    den_fold[0:fp, 0:fw],
                    den_dr[0:WW].rearrange("(p f) -> p f", f=fw))
                nc.vector.reciprocal(r_fold[0:fp, 0:fw],
                                     den_fold[0:fp, 0:fw])
                nc.sync.dma_start(
                    r_dr[0:WW].rearrange("(p f) -> p f", f=fw),
                    r_fold[0:fp, 0:fw])
                r_rep = rpool.tile([64, 2 * W], dt.float32, tag="r_rep")
                nc.sync.dma_start(r_rep[0:64, 0:WW],
                                  r_dr[None, 0:WW].to_broadcast([64, WW]))
                return r_rep

            # interleave queue: (kind, args) units issued between pairs
            def run_units(units, n):
                for _ in range(n):
                    if not units:
                        return
                    kind, args = units.pop(0)
                    if kind == 'oproj':
                        oproj_tile(*args)
                    elif kind == 'qconvd':
                        c, ks = args
                        qconv_dve(c, 32, 56, ks)
                    else:
                        qproj_unit(*args)

            def scores_pair(qs, p, tt):
                tsz = min(128, T2 - tt * 128)
                tslc = slice(tt * 128, tt * 128 + tsz)
                ps = psA.tile([128, 2 * W], dt.float32, tag="psA")
                nc.tensor.matmul(
                    ps[0:tsz, 0:W],
                    kh_sb[0:64, p, tslc],
                    qh_sb[0:64, p, qs:qs + W],
                    start=True, stop=True)
                nc.tensor.matmul(
                    ps[0:tsz, W:2 * W],
                    kh_sb[64:128, p, tslc],
                    qh_sb[64:128, p, qs:qs + W],
                    start=True, stop=True)
                return ps

            # Flat cross-pair software pipeline: the NEXT pair's first
            # scores issue before this pair's last o-accumulation, so ACT
            # exp never has a pair-boundary hole. Units (qconv/qproj/oproj,
            # all with inputs >= one pair old) drip in mid-pair.
            units = []
            all_pairs = [(bi, p) for bi in range(NB) for p in range(CT)]
            ps_s = None
            for idx, (bi, p) in enumerate(all_pairs):
                qs = bi * W
                if p == 0:
                    # band start: stage conv rows for q-proj chunk lc=bi+3,
                    # that q-proj one band later, previous band's out-proj
                    # (added last so it is consumed late, after the previous
                    # band's final evacuation has certainly landed).
                    if bi == 0:
                        units += [('qconvd', (c, ks)) for c in (0, 1)
                                  for ks in ((0, 1, 2), (3, 4, 5), (6, 7, 8))]
                    elif bi == 1:
                        units += [('qconvd', (2, ks))
                                  for ks in ((0, 1, 2), (3, 4, 5), (6, 7, 8))]
                    if 1 <= bi <= 4:
                        units += [('qproj', (bi + 2, ot)) for ot in range(CT)]
                    if bi > 0:
                        pq = (bi - 1) * W
                        units += [('oproj', (pq + i * 128, 128))
                                  for i in range(4)]

                hA, hB = 2 * p, 2 * p + 1
                ps_o = psO.tile([128, 2 * W], dt.float32, tag="psO")
                if idx == 0:
                    ps_s = scores_pair(qs, p, 0)
                for tt in range(NTT):
                    tsz = min(128, T2 - tt * 128)
                    e = epool.tile([128, 2 * W], dt.bfloat16, tag="e")
                    nc.scalar.activation(e[0:tsz, 0:2 * W],
                                         ps_s[0:tsz, 0:2 * W], AF.Exp)
                    if tt + 1 < NTT:
                        ps_s = scores_pair(qs, p, tt + 1)
                    elif idx + 1 < len(all_pairs):
                        nbi, np_ = all_pairs[idx + 1]
                        ps_s = scores_pair(nbi * W, np_, 0)
                    nc.tensor.matmul(
                        ps_o[0:65, 0:W],
                        vhT_sb[0:tsz, tt, hA * 65:hA * 65 + 65],
                        e[0:tsz, 0:W],
                        start=(tt == 0), stop=(tt == NTT - 1))
                    nc.tensor.matmul(
                        ps_o[0:65, W:2 * W],
                        vhT_sb[0:tsz, tt, hB * 65:hB * 65 + 65],
                        e[0:tsz, W:2 * W],
                        start=(tt == 0), stop=(tt == NTT - 1))
                    if tt in (1, 3):
                        run_units(units, 1)
                r_rep = norm_chain(ps_o, 2 * W)
                nc.vector.tensor_tensor(
                    out=o_sb[0:64, p, qs:qs + W],
                    in0=ps_o[0:64, 0:W],
                    in1=r_rep[0:64, 0:W], op=ALU.mult)
                nc.vector.tensor_tensor(
                    out=o_sb[64:128, p, qs:qs + W],
                    in0=ps_o[0:64, W:2 * W],
                    in1=r_rep[0:64, W:2 * W], op=ALU.mult)
                run_units(units, 1)

            # ---------------- Phase 6: 64-wide tail band ----------------
            # t-grouped scores: per pair ONE [tsz, 896] psum tile (cols
            # tt*128 + {0:64 A, 64:128 B}) -> ONE exp; shared ps_o
            # [65, 6*64] with one norm chain for all heads.
            qs, WT = NB * W, T - NB * W  # 3072, 64
            ps_o = psO.tile([128, 2 * W], dt.float32, tag="psO")
            for p in range(CT):
                hA, hB = 2 * p, 2 * p + 1
                for hh, h in ((0, hA), (1, hB)):
                    hsl = slice(64 * hh, 64 * hh + 64)
                    ps_s = psA.tile([128, 2 * W], dt.float32, tag="psA")
                    for tt in range(NTT):
                        tsz = min(128, T2 - tt * 128)
                        tslc = slice(tt * 128, tt * 128 + tsz)
                        nc.tensor.matmul(
                            ps_s[0:tsz, tt * 128:tt * 128 + WT],
                            kh_sb[hsl, p, tslc],
                            qh_sb[hsl, p, qs:qs + WT],
                            start=True, stop=True)
                    e = epool.tile([128, 2 * W], dt.bfloat16, tag="e")
                    nc.scalar.activation(e[0:128, 0:(NTT - 1) * 128],
                                         ps_s[0:128, 0:(NTT - 1) * 128],
                                         AF.Exp)
                    nc.scalar.activation(
                        e[0:16, (NTT - 1) * 128:NTT * 128],
                        ps_s[0:16, (NTT - 1) * 128:NTT * 128], AF.Exp)
                    # six head groups share one PSUM zero region (2KB bank)
                    # so each accumulation group fully closes before the
                    # next starts
                    for tt in range(NTT):
                        tsz = min(128, T2 - tt * 128)
                        nc.tensor.matmul(
                            ps_o[0:65, h * WT:h * WT + WT],
                            vhT_sb[0:tsz, tt, h * 65:h * 65 + 65],
                            e[0:tsz, tt * 128:tt * 128 + WT],
                            start=(tt == 0), stop=(tt == NTT - 1))
                # last 512-band's out-projection rides the tail band
                pq = (NB - 1) * W
                if p == 1:
                    oproj_tile(pq, 128)
                    oproj_tile(pq + 128, 128)
                elif p == 2:
                    oproj_tile(pq + 256, 128)
                    oproj_tile(pq + 384, 128)
            r_rep = norm_chain(ps_o, HEADS * WT)
            for h in range(HEADS):
                nc.vector.tensor_tensor(
                    out=o_sb[64 * (h % 2):64 * (h % 2) + 64, h // 2,
                             qs:qs + WT],
                    in0=ps_o[0:64, h * WT:h * WT + WT],
                    in1=r_rep[0:64, h * WT:h * WT + WT], op=ALU.mult)
            run_units(units, 99)
            oproj_tile(qs, WT)

    nc.compile()
    return nc


_CACHE = {}


def _prep_weights(inputs):
    import ml_dtypes
    bf16 = ml_dtypes.bfloat16
    f32 = np.float32

    def bn_fold(prefix):
        a = (np.asarray(inputs[f'bn{prefix}_s'], f32)
             / np.sqrt(np.asarray(inputs[f'bn{prefix}_v'], f32) + EPS))
        b = (np.asarray(inputs[f'bn{prefix}_b'], f32)
             - np.asarray(inputs[f'bn{prefix}_m'], f32) * a)
        return a.astype(f32), b.astype(f32)

    aq, bq = bn_fold('q')
    ak, bk = bn_fold('k')
    av, bv = bn_fold('v')

    conv_q = np.asarray(inputs['conv_q'], f32)[:, 0].reshape(DIM, 9)
    conv_k = np.asarray(inputs['conv_k'], f32)[:, 0].reshape(DIM, 9)
    conv_v = np.asarray(inputs['conv_v'], f32)[:, 0].reshape(DIM, 9)
    wq = np.asarray(inputs['wq'], f32)
    wk = np.asarray(inputs['wk'], f32)
    wv = np.asarray(inputs['wv'], f32)
    wl = np.asarray(inputs['w_last'], f32)

    qcp = np.zeros((DIM, 10), f32)
    qcp[:, :9] = conv_q * aq[:, None]
    qcp[:, 9] = bq

    wqt = np.ascontiguousarray((wq * SCALE).T).astype(bf16)  # [c, o]

    wkvt = np.stack([wk.T, wv.T], axis=1).astype(bf16)  # [c, {k,v}, o]
    kvs = np.concatenate([conv_k * ak[:, None], conv_v * av[:, None]],
                         axis=1).astype(f32)            # [c, 18]
    kvb = np.stack([bk, bv], axis=1).astype(f32)        # [c, 2]
    wlt = np.ascontiguousarray(wl.T).astype(bf16)
    idin = np.eye(128, dtype=bf16)
    return qcp, wqt, wkvt, kvs, kvb, wlt, idin


def _prep_x(xb):
    """[T, C] f32 -> zero-padded transposed [C, 58*58] bf16."""
    import ml_dtypes
    pad = np.zeros((DIM, S + 2, S + 2), np.float32)
    pad[:, 1:1 + S, 1:1 + S] = xb.T.reshape(DIM, S, S)
    return pad.reshape(DIM, (S + 2) * (S + 2)).astype(ml_dtypes.bfloat16)


def kernel(**inputs):
    from concourse.bass_utils import run_bass_kernel_spmd

    if 'nc' not in _CACHE:
        _CACHE['nc'] = build_program()
    nc = _CACHE['nc']

    qcp, wqt, wkvt, kvs, kvb, wlt, idin = _prep_weights(inputs)
    x = np.asarray(inputs['x'], np.float32)  # [8, T, C]
    B = x.shape[0]

    in_maps = []
    for b in range(B):
        in_maps.append({
            'xT': _prep_x(x[b]), 'qcp': qcp, 'wqt': wqt, 'wkvt': wkvt,
            'kvs': kvs, 'kvb': kvb, 'wlt': wlt, 'idin': idin,
        })

    res = run_bass_kernel_spmd(nc, in_maps, list(range(NCORES)))
    outs = np.stack([np.asarray(res.results[b]['out']) for b in range(B)],
                    axis=0)
    outs = outs + np.asarray(inputs['b_last'], np.float32)[None, None, :]
    return outs.astype(np.float32)
